# revision 19
# baseline (speedup 1.0000x reference)
"""Bass/Trainium2 kernel for nn_CGRE_68719477510 (ragged_sequence).

Restructure: scores[i] = X[i] . Constraints[rel(bag(i))] and the classifier
out = bag @ W.T are both projections of X onto small [53, 2070] matrices.
So one device pass computes Y = [Constraints; W] @ X.T  ([106, N]) — the only
traffic proportional to X. The segment softmax + weighted sum then operate on
the projected [N, 53] rows (P = X @ W.T), never touching X again:
    out[bag] = sum_i softmax_i(S) * P[i]  ==  (sum_i w_i X_i) @ W.T
Sharding: split sentences N=65536 into 8 contiguous chunks of 8192 (one per
core); replicate the small combined weight. The ragged segment ops run on
host over the tiny [N, 53] projection.

Precision: X and [C; W] are shipped in fp16 (e5m10). fp16xfp16 products are
exact in the f32 PSUM accumulator, so the only noise is the input rounding
(~2^-11 relative), giving ~1.7e-3 final Frobenius error — well under the
2e-2 gate — at half the DMA traffic of an f32/bf16-pair encoding. The
[106, n] result is written back as fp16 as well (scores max ~250, safely in
fp16 range).
"""

import sys

sys.path.insert(0, "/opt/trn_rl_repo")

import numpy as np

N_SENT = 65536
D_FEAT = 2070
N_REL = 53
N_CORES = 8
N_PER_CORE = N_SENT // N_CORES  # 8192
M_OUT = 2 * N_REL  # 106 rows: [Constraints; W]

KC = 128                      # contraction chunk (partition dim)
MM_N = 512                    # moving free dim per matmul (one PSUM bank)
N_KCHUNKS = (D_FEAT + KC - 1) // KC  # 17 (16x128 + 22)

XBLK = 2048                   # columns per X dma tile (512 KB, 4 KB packets)
N_XBLKS = N_PER_CORE // XBLK  # 4
# progressive sizes: the final supergroup (and so the final, unoverlappable
# writeback chunk) is small
SP_SIZES = [2048, 2048, 2048, 1024, 512, 512]
assert sum(SP_SIZES) == N_PER_CORE

_CACHE = {}


def _build_fp16():
    import concourse.mybir as mybir
    from concourse import bacc
    from concourse.tile import TileContext

    F16 = mybir.dt.float16
    F32 = mybir.dt.float32

    nc = bacc.Bacc("TRN2", target_bir_lowering=False, debug=True)
    xf = nc.dram_tensor("xf", [D_FEAT, N_PER_CORE], F16, kind="ExternalInput")
    # weights packed on host: cwf[p, k*106+m] = CW[m, 128k+p] (zero-padded)
    cwf = nc.dram_tensor("cwf", [KC, N_KCHUNKS * M_OUT], F16, kind="ExternalInput")
    yt = nc.dram_tensor("yt", [M_OUT, N_PER_CORE], F16, kind="ExternalOutput")

    # All X loads are triggered up-front: a backlogged descriptor ring
    # stripes read packets across all 16 DMA engines (~360+ GB/s); a drained
    # ring degrades to ~2 engines (~50 GB/s). HWDGE (sync/scalar) writes to
    # DRAM are pinned to engines 64/65 (~47 GB/s), but SWDGE (gpsimd) writes
    # stripe fully — so the writeback goes through gpsimd at the very end.
    X_BUFS = 44

    with TileContext(nc) as tc:
        with (
            tc.tile_pool(name="w", bufs=1) as wpool,
            tc.tile_pool(name="x", bufs=X_BUFS) as xpool,
            tc.tile_pool(name="out", bufs=1) as opool,
            tc.tile_pool(name="psum", bufs=2, space="PSUM") as ppool,
        ):
            # weights split across both X queues at the head of the rings so
            # they stripe with the early backlog instead of trickling alone
            wt = wpool.tile([KC, N_KCHUNKS * M_OUT], F16, tag="w")
            wh = (N_KCHUNKS * M_OUT) // 2
            nc.sync.dma_start(out=wt[:, :wh], in_=cwf[:, :wh])
            nc.scalar.dma_start(out=wt[:, wh:], in_=cwf[:, wh:])

            # X tiles keyed (kchunk, 1024-col block), issued in consumption
            # order; completion-semaphore recycling paces the trigger stream
            xts = {}
            qi = 0
            sp_starts = [sum(SP_SIZES[:i]) for i in range(len(SP_SIZES))]
            for sp, (c0, sz) in enumerate(zip(sp_starts, SP_SIZES)):
                b0, b1 = c0 // XBLK, (c0 + sz + XBLK - 1) // XBLK
                for k in range(N_KCHUNKS):
                    k0 = k * KC
                    kp = min(KC, D_FEAT - k0)
                    for b in range(b0, b1):
                        if (k, b) in xts:
                            continue
                        xt = xpool.tile([KC, XBLK], F16, tag="x")
                        eng = nc.sync if qi % 2 == 0 else nc.scalar
                        qi += 1
                        eng.dma_start(
                            out=xt[:kp], in_=xf[k0 : k0 + kp, b * XBLK : (b + 1) * XBLK]
                        )
                        xts[(k, b)] = xt

            out_t = opool.tile([M_OUT, N_PER_CORE], F16, tag="out")

            # expected cast-completion times (us) per supergroup, used to pace
            # the writeback chunks: the write path only sustains ~45 GB/s, so
            # chunks must trickle out through the body, small enough
            # (~110 KB ~= 2.4 us) that an X trigger recycling a chunk's
            # completion semaphore is never stalled
            wb_at = {0: 40.0, 1: 62.0, 2: 84.0, 3: 96.0, 4: 101.0, 5: None}

            for sp, (c0, sz) in enumerate(zip(sp_starts, SP_SIZES)):
                psum = ppool.tile([M_OUT, 2048], F32, tag="ps")
                for k in range(N_KCHUNKS):
                    kp = min(KC, D_FEAT - k * KC)
                    ws = slice(k * M_OUT, (k + 1) * M_OUT)
                    for s in range(sz // MM_N):
                        c = c0 + s * MM_N
                        xt = xts[(k, c // XBLK)]
                        off = c % XBLK
                        nc.tensor.matmul(
                            psum[:, s * MM_N : (s + 1) * MM_N],
                            wt[:kp, ws],
                            xt[:kp, off : off + MM_N],
                            start=(k == 0),
                            stop=(k == N_KCHUNKS - 1),
                        )
                nc.vector.tensor_copy(
                    out=out_t[:, c0 : c0 + sz], in_=psum[:, :sz]
                )
                t0 = wb_at[sp]
                for jj, cc in enumerate(range(c0, c0 + sz, MM_N)):
                    qs = slice(cc, cc + MM_N)
                    if t0 is None:
                        nc.gpsimd.dma_start(out=yt[:, qs], in_=out_t[:, qs])
                    else:
                        with tc.tile_wait_until((t0 + 3.0 * jj) / 1000.0):
                            nc.gpsimd.dma_start(out=yt[:, qs], in_=out_t[:, qs])

    nc.compile()
    return nc


def _build():
    if "fp16" not in _CACHE:
        _CACHE["fp16"] = _build_fp16()
    return _CACHE["fp16"]


def _pack_weights(CWT, dtype):
    """CWT [D_FEAT, 106] -> [128, 17*106] with wpack[p, k*106+m] = CWT[128k+p, m]."""
    pad = N_KCHUNKS * KC - D_FEAT
    cw = np.concatenate(
        [CWT.astype(np.float32), np.zeros((pad, M_OUT), dtype=np.float32)], axis=0
    )  # [2176, 106]
    return np.ascontiguousarray(
        cw.reshape(N_KCHUNKS, KC, M_OUT).transpose(1, 0, 2).reshape(KC, -1)
    ).astype(dtype)


def _ensure_ntff_hook():
    """bass_utils' trace path hard-imports antenv.axon_hooks, which this image
    lacks; shim it so a BASS_TRACE env var (or trace=True) can't crash."""
    import types

    try:
        from antenv.axon_hooks import get_axon_ntff_profile_hook  # noqa: F401

        return
    except ImportError:
        pass
    try:
        import antenv
        from trn_agent_boot.trn_boot import _ntff_profile_via_ctypes

        hook = _ntff_profile_via_ctypes("/opt/axon/libaxon_pjrt.so")
    except Exception:
        antenv, hook = None, None
    mod = types.ModuleType("antenv.axon_hooks")
    _h = [hook]
    mod.set_axon_ntff_profile_hook = lambda h: _h.__setitem__(0, h)
    mod.get_axon_ntff_profile_hook = lambda: _h[0]
    sys.modules["antenv.axon_hooks"] = mod
    if antenv is not None:
        antenv.axon_hooks = mod


def _run_device(XT, CWT, trace=False):
    """XT [D_FEAT, N_SENT] f32, CWT [D_FEAT, 106] f32 -> YT [106, N_SENT] f32."""
    _ensure_ntff_hook()
    from concourse.bass_utils import run_bass_kernel_spmd

    nc = _build()

    wpack = _pack_weights(CWT, np.float16)
    in_maps = [
        {
            "xf": np.ascontiguousarray(
                XT[:, c * N_PER_CORE : (c + 1) * N_PER_CORE]
            ).astype(np.float16),
            "cwf": wpack,
        }
        for c in range(N_CORES)
    ]

    res = run_bass_kernel_spmd(nc, in_maps, list(range(N_CORES)), trace=trace)
    yt = np.concatenate(
        [res.results[c]["yt"] for c in range(N_CORES)], axis=1
    ).astype(np.float32)
    return yt, res


def kernel(X, Constraints, W, b, X_Scope, X_Rel, _trace=False, _res_out=None):
    X = np.asarray(X)
    Constraints = np.asarray(Constraints)
    W = np.asarray(W)
    b = np.asarray(b)
    X_Scope = np.asarray(X_Scope)
    X_Rel = np.asarray(X_Rel)

    N, D = X.shape
    B = X_Scope.shape[0]
    R = Constraints.shape[0]
    assert (N, D, R) == (N_SENT, D_FEAT, N_REL), (N, D, R)

    XT = np.ascontiguousarray(X.T)
    CWT = np.ascontiguousarray(
        np.concatenate([Constraints, W], axis=0).T.astype(np.float32)
    )

    YT, res = _run_device(XT, CWT, trace=_trace)
    if _res_out is not None:
        _res_out.append(res)

    S_all = YT[:N_REL]          # [53, N] scores for every relation
    P = YT[N_REL:]              # [53, N] per-sentence classifier projections

    # host downstream on [N, 53]-sized data (mirrors reference semantics)
    starts = X_Scope[:, 0].astype(np.int64)
    seg = np.searchsorted(starts, np.arange(N, dtype=np.int64), side="right") - 1
    rel = np.asarray(X_Rel)[seg]  # wraps for seg == -1, same as jnp
    s = S_all[rel, np.arange(N)].astype(np.float64)

    valid = seg >= 0
    segv = seg[valid]
    m = np.full(B, -np.inf)
    np.maximum.at(m, segv, s[valid])
    e = np.exp(s - np.where(valid, m[np.clip(seg, 0, B - 1)], np.inf))
    e = np.where(valid, e, 0.0)
    z = np.bincount(segv, weights=e[valid], minlength=B)
    zsafe = np.where(z == 0.0, 1.0, z)
    w = e / zsafe[np.clip(seg, 0, B - 1)]

    out = np.empty((B, N_REL), dtype=np.float64)
    Pw = P.astype(np.float64) * w[None, :]
    for j in range(N_REL):
        out[:, j] = np.bincount(segv, weights=Pw[j, valid], minlength=B)
    out += b.astype(np.float64)[None, :]
    return out.astype(np.float32)


# revision 21
# speedup vs baseline: 1.0920x; 1.0920x over previous
"""Bass/Trainium2 kernel for nn_CGRE_68719477510 (ragged_sequence).

Restructure: scores[i] = X[i] . Constraints[rel(bag(i))] and the classifier
out = bag @ W.T are both projections of X onto small [53, 2070] matrices.
So one device pass computes Y = [Constraints; W] @ X.T  ([106, N]) — the only
traffic proportional to X. The segment softmax + weighted sum then operate on
the projected [N, 53] rows (P = X @ W.T), never touching X again:
    out[bag] = sum_i softmax_i(S) * P[i]  ==  (sum_i w_i X_i) @ W.T
Sharding: split sentences N=65536 into 8 contiguous chunks of 8192 (one per
core); replicate the small combined weight. The ragged segment ops run on
host over the tiny [N, 53] projection.

Precision: X and [C; W] are shipped in fp16 (e5m10). fp16xfp16 products are
exact in the f32 PSUM accumulator, so the only noise is the input rounding
(~2^-11 relative), giving ~1.7e-3 final Frobenius error — well under the
2e-2 gate — at half the DMA traffic of an f32/bf16-pair encoding. The
[106, n] result is written back as fp16 as well (scores max ~250, safely in
fp16 range).
"""

import sys

sys.path.insert(0, "/opt/trn_rl_repo")

import numpy as np

N_SENT = 65536
D_FEAT = 2070
N_REL = 53
N_CORES = 8
N_PER_CORE = N_SENT // N_CORES  # 8192
M_OUT = 2 * N_REL  # 106 rows: [Constraints; W]

KC = 128                      # contraction chunk (partition dim)
MM_N = 512                    # moving free dim per matmul (one PSUM bank)
N_KCHUNKS = (D_FEAT + KC - 1) // KC  # 17 (16x128 + 22)

XBLK = 2048                   # columns per X dma tile (512 KB, 4 KB packets)
N_XBLKS = N_PER_CORE // XBLK  # 4
# progressive sizes: the final supergroup (and so the final, unoverlappable
# writeback chunk) is small
SP_SIZES = [2048, 2048, 2048, 1024, 512, 512]
assert sum(SP_SIZES) == N_PER_CORE

_CACHE = {}


def _build_fp16():
    import concourse.mybir as mybir
    from concourse import bacc
    from concourse.tile import TileContext

    F16 = mybir.dt.float16
    F32 = mybir.dt.float32

    nc = bacc.Bacc("TRN2", target_bir_lowering=False, debug=True)
    xf = nc.dram_tensor("xf", [D_FEAT, N_PER_CORE], F16, kind="ExternalInput")
    # weights packed on host: cwf[p, k*106+m] = CW[m, 128k+p] (zero-padded)
    cwf = nc.dram_tensor("cwf", [KC, N_KCHUNKS * M_OUT], F16, kind="ExternalInput")
    yt = nc.dram_tensor("yt", [M_OUT, N_PER_CORE], F16, kind="ExternalOutput")

    # All X loads are triggered up-front: a backlogged descriptor ring
    # stripes read packets across all 16 DMA engines (~360+ GB/s); a drained
    # ring degrades to ~2 engines (~50 GB/s). HWDGE (sync/scalar) writes to
    # DRAM are pinned to engines 64/65 (~47 GB/s), but SWDGE (gpsimd) writes
    # stripe fully — so the writeback goes through gpsimd at the very end.
    X_BUFS = 44

    with TileContext(nc) as tc:
        with (
            tc.tile_pool(name="w", bufs=1) as wpool,
            tc.tile_pool(name="x", bufs=X_BUFS) as xpool,
            tc.tile_pool(name="out", bufs=1) as opool,
            tc.tile_pool(name="psum", bufs=2, space="PSUM") as ppool,
        ):
            # weights split across both X queues at the head of the rings so
            # they stripe with the early backlog instead of trickling alone
            wt = wpool.tile([KC, N_KCHUNKS * M_OUT], F16, tag="w")
            wh = (N_KCHUNKS * M_OUT) // 2
            nc.sync.dma_start(out=wt[:, :wh], in_=cwf[:, :wh])
            nc.scalar.dma_start(out=wt[:, wh:], in_=cwf[:, wh:])

            # X tiles keyed (kchunk, 1024-col block), issued in consumption
            # order; completion-semaphore recycling paces the trigger stream
            xts = {}
            qi = 0
            sp_starts = [sum(SP_SIZES[:i]) for i in range(len(SP_SIZES))]
            for sp, (c0, sz) in enumerate(zip(sp_starts, SP_SIZES)):
                b0, b1 = c0 // XBLK, (c0 + sz + XBLK - 1) // XBLK
                for k in range(N_KCHUNKS):
                    k0 = k * KC
                    kp = min(KC, D_FEAT - k0)
                    for b in range(b0, b1):
                        if (k, b) in xts:
                            continue
                        xt = xpool.tile([KC, XBLK], F16, tag="x")
                        eng = nc.sync if qi % 2 == 0 else nc.scalar
                        qi += 1
                        eng.dma_start(
                            out=xt[:kp], in_=xf[k0 : k0 + kp, b * XBLK : (b + 1) * XBLK]
                        )
                        xts[(k, b)] = xt

            out_t = opool.tile([M_OUT, N_PER_CORE], F16, tag="out")

            for sp, (c0, sz) in enumerate(zip(sp_starts, SP_SIZES)):
                psum = ppool.tile([M_OUT, 2048], F32, tag="ps")
                for k in range(N_KCHUNKS):
                    kp = min(KC, D_FEAT - k * KC)
                    ws = slice(k * M_OUT, (k + 1) * M_OUT)
                    for s in range(sz // MM_N):
                        c = c0 + s * MM_N
                        xt = xts[(k, c // XBLK)]
                        off = c % XBLK
                        nc.tensor.matmul(
                            psum[:, s * MM_N : (s + 1) * MM_N],
                            wt[:kp, ws],
                            xt[:kp, off : off + MM_N],
                            start=(k == 0),
                            stop=(k == N_KCHUNKS - 1),
                        )
                nc.vector.tensor_copy(
                    out=out_t[:, c0 : c0 + sz], in_=psum[:, :sz]
                )
                # one writeback per supergroup on the gpsimd queue: the
                # scheduler launches each at its cast time, overlapping the
                # body; only the final small chunk trails the last matmul
                nc.gpsimd.dma_start(
                    out=yt[:, c0 : c0 + sz], in_=out_t[:, c0 : c0 + sz]
                )

    nc.compile()
    return nc


def _build():
    if "fp16" not in _CACHE:
        _CACHE["fp16"] = _build_fp16()
    return _CACHE["fp16"]


def _pack_weights(CWT, dtype):
    """CWT [D_FEAT, 106] -> [128, 17*106] with wpack[p, k*106+m] = CWT[128k+p, m]."""
    pad = N_KCHUNKS * KC - D_FEAT
    cw = np.concatenate(
        [CWT.astype(np.float32), np.zeros((pad, M_OUT), dtype=np.float32)], axis=0
    )  # [2176, 106]
    return np.ascontiguousarray(
        cw.reshape(N_KCHUNKS, KC, M_OUT).transpose(1, 0, 2).reshape(KC, -1)
    ).astype(dtype)


def _ensure_ntff_hook():
    """bass_utils' trace path hard-imports antenv.axon_hooks, which this image
    lacks; shim it so a BASS_TRACE env var (or trace=True) can't crash."""
    import types

    try:
        from antenv.axon_hooks import get_axon_ntff_profile_hook  # noqa: F401

        return
    except ImportError:
        pass
    try:
        import antenv
        from trn_agent_boot.trn_boot import _ntff_profile_via_ctypes

        hook = _ntff_profile_via_ctypes("/opt/axon/libaxon_pjrt.so")
    except Exception:
        antenv, hook = None, None
    mod = types.ModuleType("antenv.axon_hooks")
    _h = [hook]
    mod.set_axon_ntff_profile_hook = lambda h: _h.__setitem__(0, h)
    mod.get_axon_ntff_profile_hook = lambda: _h[0]
    sys.modules["antenv.axon_hooks"] = mod
    if antenv is not None:
        antenv.axon_hooks = mod


def _run_device(XT, CWT, trace=False):
    """XT [D_FEAT, N_SENT] f32, CWT [D_FEAT, 106] f32 -> YT [106, N_SENT] f32."""
    _ensure_ntff_hook()
    from concourse.bass_utils import run_bass_kernel_spmd

    nc = _build()

    wpack = _pack_weights(CWT, np.float16)
    in_maps = [
        {
            "xf": np.ascontiguousarray(
                XT[:, c * N_PER_CORE : (c + 1) * N_PER_CORE]
            ).astype(np.float16),
            "cwf": wpack,
        }
        for c in range(N_CORES)
    ]

    res = run_bass_kernel_spmd(nc, in_maps, list(range(N_CORES)), trace=trace)
    yt = np.concatenate(
        [res.results[c]["yt"] for c in range(N_CORES)], axis=1
    ).astype(np.float32)
    return yt, res


def kernel(X, Constraints, W, b, X_Scope, X_Rel, _trace=False, _res_out=None):
    X = np.asarray(X)
    Constraints = np.asarray(Constraints)
    W = np.asarray(W)
    b = np.asarray(b)
    X_Scope = np.asarray(X_Scope)
    X_Rel = np.asarray(X_Rel)

    N, D = X.shape
    B = X_Scope.shape[0]
    R = Constraints.shape[0]
    assert (N, D, R) == (N_SENT, D_FEAT, N_REL), (N, D, R)

    XT = np.ascontiguousarray(X.T)
    CWT = np.ascontiguousarray(
        np.concatenate([Constraints, W], axis=0).T.astype(np.float32)
    )

    YT, res = _run_device(XT, CWT, trace=_trace)
    if _res_out is not None:
        _res_out.append(res)

    S_all = YT[:N_REL]          # [53, N] scores for every relation
    P = YT[N_REL:]              # [53, N] per-sentence classifier projections

    # host downstream on [N, 53]-sized data (mirrors reference semantics)
    starts = X_Scope[:, 0].astype(np.int64)
    seg = np.searchsorted(starts, np.arange(N, dtype=np.int64), side="right") - 1
    rel = np.asarray(X_Rel)[seg]  # wraps for seg == -1, same as jnp
    s = S_all[rel, np.arange(N)].astype(np.float64)

    valid = seg >= 0
    segv = seg[valid]
    m = np.full(B, -np.inf)
    np.maximum.at(m, segv, s[valid])
    e = np.exp(s - np.where(valid, m[np.clip(seg, 0, B - 1)], np.inf))
    e = np.where(valid, e, 0.0)
    z = np.bincount(segv, weights=e[valid], minlength=B)
    zsafe = np.where(z == 0.0, 1.0, z)
    w = e / zsafe[np.clip(seg, 0, B - 1)]

    out = np.empty((B, N_REL), dtype=np.float64)
    Pw = P.astype(np.float64) * w[None, :]
    for j in range(N_REL):
        out[:, j] = np.bincount(segv, weights=Pw[j, valid], minlength=B)
    out += b.astype(np.float64)[None, :]
    return out.astype(np.float32)


# revision 23
# speedup vs baseline: 1.1405x; 1.0444x over previous
"""Bass/Trainium2 kernel for nn_CGRE_68719477510 (ragged_sequence).

Restructure: scores[i] = X[i] . Constraints[rel(bag(i))] and the classifier
out = bag @ W.T are both projections of X onto small [53, 2070] matrices.
So one device pass computes Y = [Constraints; W] @ X.T  ([106, N]) — the only
traffic proportional to X. The segment softmax + weighted sum then operate on
the projected [N, 53] rows (P = X @ W.T), never touching X again:
    out[bag] = sum_i softmax_i(S) * P[i]  ==  (sum_i w_i X_i) @ W.T
Sharding: split sentences N=65536 into 8 contiguous chunks of 8192 (one per
core); replicate the small combined weight. The ragged segment ops run on
host over the tiny [N, 53] projection.

Precision: X and [C; W] are shipped in fp16 (e5m10). fp16xfp16 products are
exact in the f32 PSUM accumulator, so the only noise is the input rounding
(~2^-11 relative), giving ~1.7e-3 final Frobenius error — well under the
2e-2 gate — at half the DMA traffic of an f32/bf16-pair encoding. The
[106, n] result is written back as fp16 as well (scores max ~250, safely in
fp16 range).
"""

import sys

sys.path.insert(0, "/opt/trn_rl_repo")

import numpy as np

N_SENT = 65536
D_FEAT = 2070
N_REL = 53
N_CORES = 8
N_PER_CORE = N_SENT // N_CORES  # 8192
M_OUT = 2 * N_REL  # 106 rows: [Constraints; W]

KC = 128                      # contraction chunk (partition dim)
MM_N = 512                    # moving free dim per matmul (one PSUM bank)
N_KCHUNKS = (D_FEAT + KC - 1) // KC  # 17 (16x128 + 22)

XBLK = 2048                   # columns per X dma tile (512 KB, 4 KB packets)
N_XBLKS = N_PER_CORE // XBLK  # 4
SP_SIZES = [2048] * 4
assert sum(SP_SIZES) == N_PER_CORE

_CACHE = {}


def _build_fp16():
    import concourse.mybir as mybir
    from concourse import bacc
    from concourse.tile import TileContext

    F16 = mybir.dt.float16
    F32 = mybir.dt.float32

    nc = bacc.Bacc("TRN2", target_bir_lowering=False, debug=True)
    xf = nc.dram_tensor("xf", [D_FEAT, N_PER_CORE], F16, kind="ExternalInput")
    # weights packed on host: cwf[p, k*106+m] = CW[m, 128k+p] (zero-padded)
    cwf = nc.dram_tensor("cwf", [KC, N_KCHUNKS * M_OUT], F16, kind="ExternalInput")
    yt = nc.dram_tensor("yt", [M_OUT, N_PER_CORE], F16, kind="ExternalOutput")

    # All X loads are triggered up-front: a backlogged descriptor ring
    # stripes read packets across all 16 DMA engines (~360+ GB/s); a drained
    # ring degrades to ~2 engines (~50 GB/s). HWDGE (sync/scalar) writes to
    # DRAM are pinned to engines 64/65 (~47 GB/s), but SWDGE (gpsimd) writes
    # stripe fully — so the writeback goes through gpsimd at the very end.
    X_BUFS = 44

    with TileContext(nc) as tc:
        with (
            tc.tile_pool(name="w", bufs=1) as wpool,
            tc.tile_pool(name="x", bufs=X_BUFS) as xpool,
            tc.tile_pool(name="out", bufs=1) as opool,
            tc.tile_pool(name="psum", bufs=2, space="PSUM") as ppool,
        ):
            # weights split across both X queues at the head of the rings so
            # they stripe with the early backlog instead of trickling alone
            wt = wpool.tile([KC, N_KCHUNKS * M_OUT], F16, tag="w")
            wh = (N_KCHUNKS * M_OUT) // 2
            nc.sync.dma_start(out=wt[:, :wh], in_=cwf[:, :wh])
            nc.scalar.dma_start(out=wt[:, wh:], in_=cwf[:, wh:])

            # X tiles keyed (kchunk, 1024-col block), issued in consumption
            # order; completion-semaphore recycling paces the trigger stream
            xts = {}
            qi = 0
            sp_starts = [sum(SP_SIZES[:i]) for i in range(len(SP_SIZES))]
            for sp, (c0, sz) in enumerate(zip(sp_starts, SP_SIZES)):
                b0, b1 = c0 // XBLK, (c0 + sz + XBLK - 1) // XBLK
                for k in range(N_KCHUNKS):
                    k0 = k * KC
                    kp = min(KC, D_FEAT - k0)
                    for b in range(b0, b1):
                        if (k, b) in xts:
                            continue
                        xt = xpool.tile([KC, XBLK], F16, tag="x")
                        eng = nc.sync if qi % 2 == 0 else nc.scalar
                        qi += 1
                        eng.dma_start(
                            out=xt[:kp], in_=xf[k0 : k0 + kp, b * XBLK : (b + 1) * XBLK]
                        )
                        xts[(k, b)] = xt

            out_t = opool.tile([M_OUT, N_PER_CORE], F16, tag="out")

            for sp, (c0, sz) in enumerate(zip(sp_starts, SP_SIZES)):
                psum = ppool.tile([M_OUT, 2048], F32, tag="ps")
                for k in range(N_KCHUNKS):
                    kp = min(KC, D_FEAT - k * KC)
                    ws = slice(k * M_OUT, (k + 1) * M_OUT)
                    for s in range(sz // MM_N):
                        c = c0 + s * MM_N
                        xt = xts[(k, c // XBLK)]
                        off = c % XBLK
                        nc.tensor.matmul(
                            psum[:, s * MM_N : (s + 1) * MM_N],
                            wt[:kp, ws],
                            xt[:kp, off : off + MM_N],
                            start=(k == 0),
                            stop=(k == N_KCHUNKS - 1),
                        )
                nc.vector.tensor_copy(
                    out=out_t[:, c0 : c0 + sz], in_=psum[:, :sz]
                )

            # writebacks issued after the loop on the gpsimd software-DGE
            # queue (never in front of X reads in a ring; the scheduler
            # launches each as its cast completes, overlapping the body).
            # The final chunk trails the last matmul, so it is split across
            # the gpsimd path AND the (engine-64/65) HWDGE write pool, which
            # run in parallel.
            for j in range(3):
                qs = slice(j * 2048, (j + 1) * 2048)
                nc.gpsimd.dma_start(out=yt[:, qs], in_=out_t[:, qs])
            nc.gpsimd.dma_start(out=yt[:, 6144:7168], in_=out_t[:, 6144:7168])
            nc.sync.dma_start(out=yt[:, 7168:7680], in_=out_t[:, 7168:7680])
            nc.scalar.dma_start(out=yt[:, 7680:8192], in_=out_t[:, 7680:8192])

    nc.compile()
    return nc


def _build():
    if "fp16" not in _CACHE:
        _CACHE["fp16"] = _build_fp16()
    return _CACHE["fp16"]


def _pack_weights(CWT, dtype):
    """CWT [D_FEAT, 106] -> [128, 17*106] with wpack[p, k*106+m] = CWT[128k+p, m]."""
    pad = N_KCHUNKS * KC - D_FEAT
    cw = np.concatenate(
        [CWT.astype(np.float32), np.zeros((pad, M_OUT), dtype=np.float32)], axis=0
    )  # [2176, 106]
    return np.ascontiguousarray(
        cw.reshape(N_KCHUNKS, KC, M_OUT).transpose(1, 0, 2).reshape(KC, -1)
    ).astype(dtype)


def _ensure_ntff_hook():
    """bass_utils' trace path hard-imports antenv.axon_hooks, which this image
    lacks; shim it so a BASS_TRACE env var (or trace=True) can't crash."""
    import types

    try:
        from antenv.axon_hooks import get_axon_ntff_profile_hook  # noqa: F401

        return
    except ImportError:
        pass
    try:
        import antenv
        from trn_agent_boot.trn_boot import _ntff_profile_via_ctypes

        hook = _ntff_profile_via_ctypes("/opt/axon/libaxon_pjrt.so")
    except Exception:
        antenv, hook = None, None
    mod = types.ModuleType("antenv.axon_hooks")
    _h = [hook]
    mod.set_axon_ntff_profile_hook = lambda h: _h.__setitem__(0, h)
    mod.get_axon_ntff_profile_hook = lambda: _h[0]
    sys.modules["antenv.axon_hooks"] = mod
    if antenv is not None:
        antenv.axon_hooks = mod


def _run_device(XT, CWT, trace=False):
    """XT [D_FEAT, N_SENT] f32, CWT [D_FEAT, 106] f32 -> YT [106, N_SENT] f32."""
    _ensure_ntff_hook()
    from concourse.bass_utils import run_bass_kernel_spmd

    nc = _build()

    wpack = _pack_weights(CWT, np.float16)
    in_maps = [
        {
            "xf": np.ascontiguousarray(
                XT[:, c * N_PER_CORE : (c + 1) * N_PER_CORE]
            ).astype(np.float16),
            "cwf": wpack,
        }
        for c in range(N_CORES)
    ]

    res = run_bass_kernel_spmd(nc, in_maps, list(range(N_CORES)), trace=trace)
    yt = np.concatenate(
        [res.results[c]["yt"] for c in range(N_CORES)], axis=1
    ).astype(np.float32)
    return yt, res


def kernel(X, Constraints, W, b, X_Scope, X_Rel, _trace=False, _res_out=None):
    X = np.asarray(X)
    Constraints = np.asarray(Constraints)
    W = np.asarray(W)
    b = np.asarray(b)
    X_Scope = np.asarray(X_Scope)
    X_Rel = np.asarray(X_Rel)

    N, D = X.shape
    B = X_Scope.shape[0]
    R = Constraints.shape[0]
    assert (N, D, R) == (N_SENT, D_FEAT, N_REL), (N, D, R)

    XT = np.ascontiguousarray(X.T)
    CWT = np.ascontiguousarray(
        np.concatenate([Constraints, W], axis=0).T.astype(np.float32)
    )

    YT, res = _run_device(XT, CWT, trace=_trace)
    if _res_out is not None:
        _res_out.append(res)

    S_all = YT[:N_REL]          # [53, N] scores for every relation
    P = YT[N_REL:]              # [53, N] per-sentence classifier projections

    # host downstream on [N, 53]-sized data (mirrors reference semantics)
    starts = X_Scope[:, 0].astype(np.int64)
    seg = np.searchsorted(starts, np.arange(N, dtype=np.int64), side="right") - 1
    rel = np.asarray(X_Rel)[seg]  # wraps for seg == -1, same as jnp
    s = S_all[rel, np.arange(N)].astype(np.float64)

    valid = seg >= 0
    segv = seg[valid]
    m = np.full(B, -np.inf)
    np.maximum.at(m, segv, s[valid])
    e = np.exp(s - np.where(valid, m[np.clip(seg, 0, B - 1)], np.inf))
    e = np.where(valid, e, 0.0)
    z = np.bincount(segv, weights=e[valid], minlength=B)
    zsafe = np.where(z == 0.0, 1.0, z)
    w = e / zsafe[np.clip(seg, 0, B - 1)]

    out = np.empty((B, N_REL), dtype=np.float64)
    Pw = P.astype(np.float64) * w[None, :]
    for j in range(N_REL):
        out[:, j] = np.bincount(segv, weights=Pw[j, valid], minlength=B)
    out += b.astype(np.float64)[None, :]
    return out.astype(np.float32)
